# revision 51
# baseline (speedup 1.0000x reference)
"""GAT-style graph-attention kernel for Trainium2, sharded over 8 NeuronCores.

Math (reference):
  h = x*conv_w + conv_b                       [N, D]
  Wh1 = h @ a1.T ; Wh2 = h @ a2.T             [N, H]
  e[k,i,j] = elu(Wh1[i,k] + Wh2[j,k])
  att = softmax_j(where(adj>0, e, -9e15))
  out = elu(0.5*mean_k(att@h) + 0.5*h); out /= max(||out||_2, 1e-12); out += bias

Device identities:
  q = e^z = e^{w1_i} * e^{w2_j}  (rank-1 outer product, built on the PE)
  p := exp(elu(z)) * mask = (min(e^{q-1}, max(q, 1))) * mask
  With t = q - 1 + Madd (Madd = 0 unmasked, -BIG masked) this whole chain is
  ONE custom 8-stage DVE op:
      p = min(A*t^2 + B*t, relu(t)) + (t > -BIG/2)
  where A*t^2 + B*t ~= e^t - 1 on [-1, 0] (max rel err 6.8e-3), the relu term
  realises max(q,1)-1, the compare adds back the +1 only for unmasked lanes
  (masked lanes: poly>0 huge, relu=0 -> min=0, cmp=0 -> p=0 exactly).
  Softmax denominators ride a ones-column appended to h in the att@h
  matmul (the row-sum is a free extra output column).

All setup products (h_aug, exp(Wh1), exp(Wh2), broadcasts, additive masks)
are precomputed on the host so the device runs only the O(N^2) part.
Sharding: each core owns a 512-row block of the output for all 4 heads
(row-parallel, no collectives)."""
import sys

if "/opt/trn_rl_repo" not in sys.path:
    sys.path.insert(0, "/opt/trn_rl_repo")

import numpy as np
from contextlib import ExitStack

import concourse.tile as tile
from concourse import bacc, mybir
from concourse import dve_ops
from concourse.dve_spec import (Src0, Src1, C0, C1, C2, Bin, AluOp, relu,
                                minn, Spec, lower)
from concourse.dve_uop import DveOpSpec

N, D, H = 4096, 256, 4
NCORES = 8
R = N // NCORES          # 512 rows per core
JT = N // 128            # 32 j-tiles
IC = R // 128            # 4 i-chunks per core
SB = 2                   # j-tiles per superblock
NSB = JT // SB           # 16 superblocks
WID = SB * R             # free width of a score tile (1024)
MG = 4                   # mask DMA groups

# e^t - 1 ~= A t^2 + B t on [-1, 0] (minimax in relative error, 6.8e-3)
POLY_A = 0.31220335810677635
POLY_B = 0.94183886395738
BIG = 1e20
CMP_TH = -5e19

FP32 = mybir.dt.float32
BF16 = mybir.dt.bfloat16
AF = mybir.ActivationFunctionType
ALU = mybir.AluOpType


def _register_gat_op():
    """Build + register the fused score op with the custom-DVE registry."""
    name = "GAT_SCORE_ANT"
    for op in dve_ops.OPS:
        if op.name == name:
            return op
    t = Bin(AluOp.ADD, Src0, Src1)
    poly0 = Bin(AluOp.MULTIPLY,
                Bin(AluOp.ADD, Bin(AluOp.MULTIPLY, C0, t), C1), t)
    body = Bin(AluOp.ADD, minn(poly0, relu(t)), Bin(AluOp.IS_GT, t, C2))

    def ref(in0, in1, s0, s1, imm2):
        with np.errstate(over="ignore", invalid="ignore"):
            tt = (in0.astype(np.float32) + in1.astype(np.float32))
            p0 = ((np.float32(s0) * tt + np.float32(s1)) * tt).astype(np.float32)
            out = (np.minimum(p0, np.maximum(tt, np.float32(0.0)))
                   + (tt > np.float32(imm2)).astype(np.float32))
        return out.astype(np.float32)

    spec = Spec(body=body, reference=ref)
    shas = {}
    for ver in ("v3", "v4"):
        try:
            s = DveOpSpec(name=name, opcode=0, uops=lower(spec, ver=ver),
                          rd1_en=True)
            shas[ver] = s.sha(ver)
        except Exception:
            pass
    op = dve_ops.DveOp(name, spec, subdim=False, uops_sha=shas)
    dve_ops.OPS.append(op)
    dve_ops._SUB_OPCODE_FOR_NAME[name] = (dve_ops._CUSTOM_DVE_ROW_BASE
                                          + len(dve_ops.OPS) - 1)
    dve_ops.CUSTOM_DVE_SPECS[name] = spec
    return op


GAT_OP = _register_gat_op()


def _build_program():
    nc = bacc.Bacc("TRN2", target_bir_lowering=False, debug=False,
                   num_devices=NCORES)

    haug_d = nc.dram_tensor("haug", [128, JT * (D + 1)], BF16,
                            kind="ExternalInput")
    hI_d = nc.dram_tensor("hI", [128, IC * D], FP32, kind="ExternalInput")
    biasb_d = nc.dram_tensor("biasb", [128, D], FP32, kind="ExternalInput")
    ew_d = nc.dram_tensor("ew", [1, H * (N + R)], BF16,
                          kind="ExternalInput")
    mp_d = nc.dram_tensor("mp", [128, JT * R], BF16, kind="ExternalInput")
    out_d = nc.dram_tensor("out", [R, D], FP32, kind="ExternalOutput")

    with tile.TileContext(nc) as tc, ExitStack() as ctx:
        per = ctx.enter_context(tc.tile_pool(name="per", bufs=1))
        ewall = per.tile([1, H * (N + R)], BF16, tag="ewall")
        haug = per.tile([128, JT * (D + 1)], BF16, tag="haug")
        mt = [per.tile([128, (JT // MG) * R], BF16, tag=f"m{g}", name=f"m{g}")
              for g in range(MG)]
        h_I = per.tile([128, IC * D], FP32, tag="h_I")
        hIacc = per.tile([128, IC * D], FP32, tag="hIacc")
        accp = per.tile([128, IC * D], FP32, tag="accp")
        bias_bc = per.tile([128, D], FP32, tag="bias_bc")
        neg1 = per.tile([128, 1], FP32, tag="neg1")

        pp = ctx.enter_context(tc.tile_pool(name="p", bufs=6))
        ep = ctx.enter_context(tc.tile_pool(name="ep", bufs=8))
        epi = ctx.enter_context(tc.tile_pool(name="epi", bufs=1))

        # ---------------- input DMAs (tiny first: unblock the main loop) ---
        nc.sync.dma_start(ewall[:], ew_d[:, :])
        nc.gpsimd.memset(neg1[:], -1.0)
        # one FIFO DMA queue, ordered by first-use time in sweep 0
        gw = (JT // MG) * R
        hq = JT // MG * (D + 1)

        # preload the ACT function table (Sqrt then Exp set) immediately
        warm = ep.tile([128, 1], FP32, tag="warm", name="warm")
        nc.scalar.activation(warm[:], neg1[:], AF.Sqrt)
        nc.scalar.activation(warm[:], warm[:], AF.Exp)

        def m_chunk(g, qq, nq):
            cw = gw // nq
            nc.sync.dma_start(
                mt[g][:, qq * cw:(qq + 1) * cw],
                mp_d[:, g * gw + qq * cw:g * gw + (qq + 1) * cw])

        def h_chunk(g):
            nc.sync.dma_start(haug[:, g * hq:(g + 1) * hq],
                              haug_d[:, g * hq:(g + 1) * hq])

        for qq in range(4):
            m_chunk(0, qq, 4)
        h_chunk(0)
        m_chunk(1, 0, 1)
        h_chunk(1)
        m_chunk(2, 0, 1)
        h_chunk(2)
        m_chunk(3, 0, 1)
        h_chunk(3)
        # (order above: each chunk lands >=2us before its first use in
        # sweep 0 at the observed ~1.28us/superblock consumption rate);
        # epilogue-only tensors come last so they never steal DMA slots
        nc.sync.dma_start(h_I[:], hI_d[:, :])
        nc.sync.dma_start(bias_bc[:], biasb_d[:, :])
        
        def haug_sl(jb):
            return haug[:, jb * (D + 1):(jb + 1) * (D + 1)]

        # ---------------- main: one flattened pipeline over 4 head sweeps ---
        # 3-stage software pipeline: q(g) on PE two iterations ahead of its
        # matmuls so the in-order PE queue never blocks the DVE. Sweeps are
        # chained without draining; pm PSUM is freed by split fold-copies
        # (2 on DVE, 2 on ACT) right after each sweep's last matmul.
        G = H * NSB

        def fold(k, pm):
            if k == H - 1:
                return  # merged into the epilogue (reads pm from PSUM)
            for icc in range(IC):
                pmc = ep.tile([128, D + 1], FP32, tag=f"pmc{icc % 2}",
                              name=f"pmc{k}_{icc}")
                if icc % 2 == 0:
                    nc.vector.tensor_copy(pmc[:], pm[icc][:])
                else:
                    nc.scalar.activation(pmc[:], pm[icc][:], AF.Copy)
                rcp = ep.tile([128, 1], FP32, tag="rcp",
                              name=f"rcp{k}_{icc}")
                nc.vector.reciprocal(rcp[:], pmc[:, D:D + 1])
                acs = accp[:, icc * D:(icc + 1) * D]
                if k == 0:
                    nc.gpsimd.tensor_scalar(acs, pmc[:, :D], rcp[:],
                                            None, op0=ALU.mult)
                else:
                    tmp = ep.tile([128, D], FP32, tag="ftmp",
                                  name=f"ftmp{k}_{icc}")
                    nc.gpsimd.tensor_scalar(tmp[:], pmc[:, :D], rcp[:],
                                            None, op0=ALU.mult)
                    nc.gpsimd.tensor_add(acs, acs, tmp[:])
            if k == H - 2:
                # prebuild 0.125*accp + 0.5*h on the idle gpsimd; hides in
                # sweep 3 and lets the final fold fuse into the epilogue
                for icc in range(IC):
                    sl = slice(icc * D, (icc + 1) * D)
                    nc.gpsimd.tensor_scalar(hIacc[:, sl], accp[:, sl],
                                            0.125, None, op0=ALU.mult)
                    nc.gpsimd.tensor_add(hIacc[:, sl], hIacc[:, sl],
                                         h_I[:, sl])

        with tc.tile_pool(name="pm", bufs=1, space="PSUM") as pmp, \
             tc.tile_pool(name="qps", bufs=2, space="PSUM") as qpp:
            pm = None
            qtiles = {}
            ptiles = {}
            for step in range(G + 2):
                if step < G:
                    k, s = divmod(step, NSB)
                    q = qpp.tile([128, WID], FP32, tag="q",
                                 name=f"q{k}_{s}")
                    qtiles[step] = q
                    for jl in range(SB):
                        jb = s * SB + jl
                        nc.tensor.matmul(
                            q[:, jl * R:(jl + 1) * R],
                            ewall[:, k * N + jb * 128:k * N + (jb + 1) * 128],
                            ewall[:, H * N + k * R:H * N + (k + 1) * R],
                            start=True, stop=True)
                if 1 <= step <= G:
                    k, s = divmod(step - 1, NSB)
                    msec = mt[s // (NSB // MG)][
                        :, (s % (NSB // MG)) * WID:
                        (s % (NSB // MG) + 1) * WID]
                    p = pp.tile([128, WID], BF16, tag="p",
                                name=f"p{k}_{s}")
                    ptiles[step - 1] = p
                    q = qtiles.pop(step - 1)
                    nc.vector._custom_dve(
                        GAT_OP, out=p[:], in0=q[:], in1=msec,
                        s0=POLY_A, s1=POLY_B, imm2=CMP_TH)
                if step >= 2:
                    k, s = divmod(step - 2, NSB)
                    if s == 0:
                        pm = [pmp.tile([128, D + 1], FP32, tag=f"pm{icc}",
                                       name=f"pm{icc}_{k}")
                              for icc in range(IC)]
                    p = ptiles.pop(step - 2)
                    if s == NSB - 1:
                        # icc-major so pm[icc] finishes (and folds) earlier
                        for icc in range(IC):
                            for jl in range(SB):
                                jb = s * SB + jl
                                nc.tensor.matmul(
                                    pm[icc][:],
                                    p[:, jl * R + icc * 128:
                                      jl * R + (icc + 1) * 128],
                                    haug_sl(jb),
                                    start=False, stop=(jl == SB - 1))
                    else:
                        for jl in range(SB):
                            jb = s * SB + jl
                            rhs = haug_sl(jb)
                            for icc in range(IC):
                                nc.tensor.matmul(
                                    pm[icc][:],
                                    p[:, jl * R + icc * 128:
                                      jl * R + (icc + 1) * 128],
                                    rhs,
                                    start=(s == 0 and jl == 0),
                                    stop=False)
                    if s == NSB - 1:
                        fold(k, pm)

            # ---------------- epilogue (phase-batched across i-chunks) ---
            ts, os_, sss, rcps = [], [], [], []
            for icc in range(IC):
                den8 = epi.tile([128, 1], FP32, tag=f"d8{icc}",
                                name=f"d8{icc}")
                nc.vector.tensor_scalar(den8[:], pm[icc][:, D:D + 1], 8.0,
                                        None, op0=ALU.mult)
                rcp8 = epi.tile([128, 1], FP32, tag=f"r8{icc}",
                                name=f"r8{icc}")
                nc.vector.reciprocal(rcp8[:], den8[:])
                t = epi.tile([128, D], FP32, tag=f"t{icc}", name=f"t{icc}")
                nc.vector.scalar_tensor_tensor(
                    t[:], pm[icc][:, :D], rcp8[:],
                    hIacc[:, icc * D:(icc + 1) * D],
                    op0=ALU.mult, op1=ALU.add)
                ts.append(t)
            eqs = []
            for icc in range(IC):
                eq = epi.tile([128, D], FP32, tag=f"eq{icc}", name=f"eq{icc}")
                nc.scalar.activation(eq[:], ts[icc][:], AF.Exp)
                eqs.append(eq)
            o1s = []
            for icc in range(IC):
                o1 = epi.tile([128, D], FP32, tag=f"o1{icc}", name=f"o1{icc}")
                nc.gpsimd.tensor_scalar(o1[:], eqs[icc][:], 1.0, -1.0,
                                        op0=ALU.min, op1=ALU.add)
                o1s.append(o1)
            for icc in range(IC):
                o = epi.tile([128, D], FP32, tag=f"o{icc}", name=f"o{icc}")
                nc.vector.scalar_tensor_tensor(o[:], ts[icc][:], 0.0,
                                               o1s[icc][:],
                                               op0=ALU.max, op1=ALU.add)
                os_.append(o)
            for icc in range(IC):
                sq = epi.tile([128, D], FP32, tag="sq", name=f"sq{icc}")
                ss = epi.tile([128, 1], FP32, tag=f"ss{icc}", name=f"ss{icc}")
                nc.scalar.activation(sq[:], os_[icc][:], AF.Square,
                                     accum_out=ss[:])
                sss.append(ss)
            nrms = []
            for icc in range(IC):
                nrm = epi.tile([128, 1], FP32, tag=f"nrm{icc}",
                              name=f"nrm{icc}")
                nc.scalar.activation(nrm[:], sss[icc][:], AF.Sqrt)
                nrms.append(nrm)
            for icc in range(IC):
                nrm2 = epi.tile([128, 1], FP32, tag=f"n2{icc}",
                               name=f"n2{icc}")
                nc.vector.tensor_scalar(nrm2[:], nrms[icc][:], 1e-12, None,
                                        op0=ALU.max)
                rcpn = epi.tile([128, 1], FP32, tag=f"rc{icc}",
                               name=f"rc{icc}")
                nc.vector.reciprocal(rcpn[:], nrm2[:])
                rcps.append(rcpn)
            for icc in range(IC):
                outv = epi.tile([128, D], FP32, tag=f"ov{icc}",
                               name=f"ov{icc}")
                nc.vector.scalar_tensor_tensor(
                    outv[:], os_[icc][:], rcps[icc][:], bias_bc[:],
                    op0=ALU.mult, op1=ALU.add)
                eng = nc.sync if icc % 2 == 0 else nc.scalar
                eng.dma_start(out_d[icc * 128:(icc + 1) * 128, :], outv[:])

    nc.finalize()
    return nc


_PROGRAM_CACHE = {}


def _get_program():
    if "p" not in _PROGRAM_CACHE:
        _PROGRAM_CACHE["p"] = _build_program()
    return _PROGRAM_CACHE["p"]


def _tile128(arr2d, blk):
    """[T*128, W] -> [128, T*W] with tile-major free dim (blk = W)."""
    t = arr2d.shape[0] // 128
    return np.ascontiguousarray(
        arr2d.reshape(t, 128, blk).transpose(1, 0, 2).reshape(128, t * blk))


def kernel(x, adj, conv_w, conv_b, a, bias, _want_results=False, _trace=False,
           **_ignored):
    import ml_dtypes
    from concourse.bass_utils import run_bass_kernel_spmd

    bf16 = ml_dtypes.bfloat16
    x = np.asarray(x, dtype=np.float32)
    adj = np.asarray(adj)
    a = np.asarray(a, dtype=np.float32)
    bias = np.asarray(bias, dtype=np.float32)
    w_conv = float(np.asarray(conv_w).reshape(-1)[0])
    b_conv = float(np.asarray(conv_b).reshape(-1)[0])

    xn = np.ascontiguousarray(x.reshape(N, D))
    h = w_conv * xn + b_conv                               # [N, D] fp32
    a1 = a[:, :D, 0]
    a2 = a[:, D:, 0]
    Wh1 = h @ a1.T                                         # [N, H]
    Wh2 = h @ a2.T
    ew1 = np.exp(Wh1).astype(bf16)                         # [N, H]
    ew2 = np.exp(Wh2).astype(bf16)

    haug_full = np.concatenate(
        [h, np.ones((N, 1), np.float32)], axis=1).astype(bf16)  # [N, 257]
    haug = _tile128(haug_full, D + 1)
    ew2r = np.ascontiguousarray(ew2.T)                     # [H, N]
    # per-partition w2-exp scalars: [128, (jt, head)]
    qwh = np.ascontiguousarray(
        ew2.reshape(JT, 128, H).transpose(1, 0, 2).reshape(128, JT * H))
    madd = np.where(adj > 0, np.float32(-1.0), np.float32(-BIG))

    nc = _get_program()

    in_maps = []
    for c in range(NCORES):
        rows = slice(c * R, (c + 1) * R)
        mT = np.ascontiguousarray(madd[rows].T)            # [N, R]
        mp = _tile128(mT.astype(bf16), R)                  # [128, JT*R]
        ew1c = np.ascontiguousarray(ew1[rows].T)           # [H, R]
        qv1 = np.broadcast_to(ew1c.reshape(1, H * R),
                              (128, H * R)).astype(bf16)
        hI = (0.5 * h[rows]).astype(np.float32)            # [R, D]
        ew = np.concatenate([ew2r.reshape(-1), ew1c.reshape(-1)])
        in_maps.append({
            "haug": haug,
            "hI": _tile128(hI, D),
            "biasb": np.broadcast_to(bias.reshape(1, D),
                                     (128, D)).astype(np.float32),
            "ew": np.ascontiguousarray(ew.reshape(1, -1)).astype(bf16),
            "mp": mp,
        })

    res = run_bass_kernel_spmd(nc, in_maps, core_ids=list(range(NCORES)),
                               trace=_trace)
    out = np.concatenate([res.results[c]["out"] for c in range(NCORES)],
                         axis=0)
    if _want_results:
        return out, res
    return out
